# revision 1
# baseline (speedup 1.0000x reference)
"""Trainium2 Bass kernel for gated GQA attention (Qwen3.5-style block).

Full inputs -> full outputs. Internally shards batch over 4 cores (fsdp) x
heads over 2 cores (tp): core c handles batch c//2, head-half c%2
(16 q-heads / 4 kv-heads). Host sums the two tp partial outputs per batch.

All matmuls run in float32r (tf32-like). Layouts are chosen so no on-device
transposes are needed:
  - q/k projected to [head_dim, token] (weight tiles stationary)
  - v projected to [token, head_dim]
  - scores computed transposed: scoresT[s, t] = sum_h k[h,s] q[h,t]
  - softmax without max-subtraction (scores are O(10)); denominator via
    ones-vector matmul over the partition dim, reciprocal broadcast back
    with a rank-1 ones matmul
  - RoPE via a constant 128x128 swap/sign matrix matmul plus host-built
    cos/sin tables; rms-norm (1+w) and the 1/sqrt(H) score scale are folded
    into the tables / norm factors on the host
"""
import sys, os
sys.path.insert(0, '/opt/trn_rl_repo')
from contextlib import ExitStack

import numpy as np

import concourse.bass as bass
import concourse.tile as tile
from concourse import bacc, mybir
from concourse.bass_utils import run_bass_kernel_spmd

F32R = mybir.dt.float32r
F32 = mybir.dt.float32
F16 = mybir.dt.float16
AF = mybir.ActivationFunctionType

B, T, D, N, K, H = 4, 1024, 4096, 32, 8, 128
ROT = 80          # rotary dims per head
RH = ROT // 2     # 40
THETA = 1.0e6
EPS = 1e-6
NL = N // 2       # 16 q heads per core
KL = K // 2       # 4 kv heads per core
NKT = D // 128    # 32 contraction tiles
TQ = 256          # token tile for attention columns
NCOL = T // TQ    # 4
NTT = T // 128    # 8 token tiles of 128
ND = D // 512     # 8 output d tiles

N_CORES = 8

_NC_CACHE = {}


class _PhaseStop(Exception):
    pass


def build_nc():
    if "nc" in _NC_CACHE:
        return _NC_CACHE["nc"]
    nc = bacc.Bacc("TRN2", target_bir_lowering=False, debug=False)

    # ---- DRAM I/O ----
    xtp = nc.dram_tensor("xtp", [NKT, 128, T], F32R, kind="ExternalInput").ap()
    wqp = nc.dram_tensor("wqp", [NL, 2, 128, NKT, 128], F32R, kind="ExternalInput").ap()
    wkp = nc.dram_tensor("wkp", [KL, 128, NKT, 128], F32R, kind="ExternalInput").ap()
    wvp = nc.dram_tensor("wvp", [NKT, 128, KL * 128], F32R, kind="ExternalInput").ap()
    wop = nc.dram_tensor("wop", [ND, 128, NL, 512], F32R, kind="ExternalInput").ap()
    cosq = nc.dram_tensor("cosq", [128, T], F32, kind="ExternalInput").ap()
    sinq = nc.dram_tensor("sinq", [ROT, T], F32, kind="ExternalInput").ap()
    cosk = nc.dram_tensor("cosk", [128, T], F32, kind="ExternalInput").ap()
    sink = nc.dram_tensor("sink", [ROT, T], F32, kind="ExternalInput").ap()
    pswap = nc.dram_tensor("pswap", [128, 128], F32R, kind="ExternalInput").ap()
    masks = nc.dram_tensor("masks", [2, 128, TQ], F32, kind="ExternalInput").ap()
    onec = nc.dram_tensor("onec", [128, 1], F32R, kind="ExternalInput").ap()
    oner = nc.dram_tensor("oner", [1, 128], F32R, kind="ExternalInput").ap()
    epsq = nc.dram_tensor("epsq", [1, 1], F32, kind="ExternalInput").ap()
    epsk = nc.dram_tensor("epsk", [1, 1], F32, kind="ExternalInput").ap()
    out = nc.dram_tensor("out", [T, D], F32, kind="ExternalOutput").ap()

    with tile.TileContext(nc) as tc:
      try:
        with ExitStack() as ctx:
            # ---- constants (live whole kernel) ----
            cpool = ctx.enter_context(tc.tile_pool(name="consts", bufs=1))
            c_pswap = cpool.tile([128, 128], F32R)
            nc.sync.dma_start(c_pswap, pswap)
            c_onec = cpool.tile([128, 1], F32R)
            nc.sync.dma_start(c_onec, onec)
            c_oner = cpool.tile([1, 128], F32R)
            nc.sync.dma_start(c_oner, oner)
            c_m0 = cpool.tile([128, TQ], F32)
            nc.sync.dma_start(c_m0, masks[0])
            c_m1 = cpool.tile([128, TQ], F32)
            nc.sync.dma_start(c_m1, masks[1])
            c_epsq = cpool.tile([1, 1], F32)
            nc.sync.dma_start(c_epsq, epsq)
            c_epsk = cpool.tile([1, 1], F32)
            nc.sync.dma_start(c_epsk, epsk)

            # ---- DRAM scratch ----
            dpool = ctx.enter_context(tc.tile_pool(name="scratch", bufs=1, space="DRAM"))
            qf_d = dpool.tile([NL, 128, T], F32R)
            kf_d = dpool.tile([KL, 128, T], F32R)
            gate_d = dpool.tile([NL, 128, T], F16)
            v_d = dpool.tile([NTT, 128, KL * 128], F32R)

            # ================= PHASE 1: projections =================
            with ExitStack() as p1:
                xpool = p1.enter_context(tc.tile_pool(name="xt", bufs=NKT))
                tabpool = p1.enter_context(tc.tile_pool(name="tables", bufs=1))
                wpool = p1.enter_context(tc.tile_pool(name="wslab", bufs=3))
                rpool = p1.enter_context(tc.tile_pool(name="rope", bufs=2))
                gpool = p1.enter_context(tc.tile_pool(name="gdrain", bufs=3))

                c_cosq = tabpool.tile([128, T], F32)
                nc.sync.dma_start(c_cosq, cosq)
                c_sinq = tabpool.tile([ROT, T], F32)
                nc.sync.dma_start(c_sinq, sinq)
                c_cosk = tabpool.tile([128, T], F32)
                nc.sync.dma_start(c_cosk, cosk)
                c_sink = tabpool.tile([ROT, T], F32)
                nc.sync.dma_start(c_sink, sink)

                xt = []
                for j in range(NKT):
                    xj = xpool.tile([128, T], F32R, tag="xt")
                    nc.sync.dma_start(xj, xtp[j])
                    xt.append(xj)

                def proj_feat_tile(w_slab_dram, psum_pool):
                    """Accumulate one 128-feature projection tile over all of D.
                    Returns two [128,512] psum tiles (token halves)."""
                    slabs = []
                    for qr in range(4):
                        ws = wpool.tile([128, 8, 128], F32R, tag="wslab")
                        nc.sync.dma_start(ws, w_slab_dram[:, qr * 8:(qr + 1) * 8, :])
                        slabs.append(ws)
                    psA = psum_pool.tile([128, 512], F32, tag="psq", bufs=4)
                    psB = psum_pool.tile([128, 512], F32, tag="psq", bufs=4)
                    for j in range(NKT):
                        wt = slabs[j // 8][:, j % 8, :]
                        nc.tensor.matmul(psA, wt, xt[j][:, 0:512],
                                         start=(j == 0), stop=(j == NKT - 1))
                        nc.tensor.matmul(psB, wt, xt[j][:, 512:1024],
                                         start=(j == 0), stop=(j == NKT - 1))
                    return psA, psB

                def rope_norm_drain(psA, psB, c_cos, c_sin, q_scale, dst, psum_pool):
                    """RMS-norm + RoPE a [128, T] projected head; write to scratch."""
                    sbq = rpool.tile([128, T], F32R, tag="sbq")
                    nc.scalar.activation(sbq[:, 0:512], psA, AF.Copy)
                    nc.scalar.activation(sbq[:, 512:1024], psB, AF.Copy)
                    for hf in range(2):
                        sl = slice(hf * 512, hf * 512 + 512)
                        q2h = rpool.tile([128, 512], F32R, tag="q2h")
                        nc.vector.tensor_mul(q2h, sbq[:, sl], sbq[:, sl])
                        ps_ss = psum_pool.tile([1, 512], F32, tag="pssum", bufs=1)
                        nc.tensor.matmul(ps_ss, c_onec, q2h, start=True, stop=True)
                        sqv = rpool.tile([1, 512], F32, tag="sqv")
                        if q_scale:
                            # 1/sqrt(sumsq + H*eps) == H^-0.5 * rsqrt(mean+eps)
                            nc.scalar.activation(sqv, ps_ss, AF.Sqrt, bias=c_epsq)
                        else:
                            nc.scalar.activation(sqv, ps_ss, AF.Sqrt, bias=c_epsk,
                                                 scale=float(1.0 / H))
                        rr = rpool.tile([1, 512], F32R, tag="rr")
                        with nc.allow_low_precision(reason="f32r output is f32-width"):
                            nc.vector.reciprocal(rr, sqv)
                        ps_qp = psum_pool.tile([128, 512], F32, tag="pqp", bufs=3)
                        nc.tensor.matmul(ps_qp, c_pswap, sbq[:, sl], start=True, stop=True)
                        ps_rb = psum_pool.tile([128, 512], F32, tag="pqp", bufs=3)
                        nc.tensor.matmul(ps_rb, c_oner, rr, start=True, stop=True)
                        qfh = rpool.tile([128, 512], F32R, tag="qfh")
                        nc.vector.tensor_mul(qfh, sbq[:, sl], c_cos[:, sl])
                        t2h = rpool.tile([ROT, 512], F32, tag="t2h")
                        nc.vector.tensor_mul(t2h, ps_qp[0:ROT], c_sin[:, sl])
                        nc.vector.tensor_add(qfh[0:ROT], qfh[0:ROT], t2h)
                        nc.vector.tensor_mul(qfh, qfh, ps_rb)
                        nc.sync.dma_start(dst[:, sl], qfh)

                # --- phase 1a: q / gate / k projections ---
                with ExitStack() as p1a:
                    psum1 = p1a.enter_context(
                        tc.tile_pool(name="psum1", bufs=3, space="PSUM"))
                    for n in range(NL):
                        psA, psB = proj_feat_tile(wqp[n, 0], psum1)
                        rope_norm_drain(psA, psB, c_cosq, c_sinq, True, qf_d[n], psum1)
                        psA, psB = proj_feat_tile(wqp[n, 1], psum1)
                        gh0 = gpool.tile([128, 512], F16, tag="gh")
                        nc.scalar.activation(gh0, psA, AF.Sigmoid)
                        nc.sync.dma_start(gate_d[n][:, 0:512], gh0)
                        gh1 = gpool.tile([128, 512], F16, tag="gh")
                        nc.scalar.activation(gh1, psB, AF.Sigmoid)
                        nc.sync.dma_start(gate_d[n][:, 512:1024], gh1)
                    for kv in range(KL):
                        psA, psB = proj_feat_tile(wkp[kv], psum1)
                        rope_norm_drain(psA, psB, c_cosk, c_sink, False, kf_d[kv], psum1)

                # --- phase 1b: v projection (form A) ---
                with ExitStack() as p1b:
                    psumv = p1b.enter_context(
                        tc.tile_pool(name="psumv", bufs=NTT, space="PSUM"))
                    vwpool = p1b.enter_context(tc.tile_pool(name="vw", bufs=4))
                    psv = []
                    for tt in range(NTT):
                        pv = psumv.tile([128, KL * 128], F32, tag="psv")
                        psv.append(pv)
                    for j in range(NKT):
                        wv_t = vwpool.tile([128, KL * 128], F32R, tag="wv")
                        nc.sync.dma_start(wv_t, wvp[j])
                        for tt in range(NTT):
                            nc.tensor.matmul(psv[tt], xt[j][:, tt * 128:(tt + 1) * 128],
                                             wv_t, start=(j == 0), stop=(j == NKT - 1))
                    for tt in range(NTT):
                        vsb = gpool.tile([128, KL * 128], F32R, tag="vsb")
                        nc.scalar.activation(vsb, psv[tt], AF.Copy)
                        nc.sync.dma_start(v_d[tt], vsb)

            _PHASES = int(os.environ.get("KM_PHASES", "3"))
            # ================= PHASE 2: attention =================
            if _PHASES < 2:
                raise _PhaseStop()
            # attn tiles live in SBUF from phase 2 into phase 3 (allocated
            # after phase 1 frees its pools)
            apool = ctx.enter_context(tc.tile_pool(name="attn", bufs=NL * NCOL))
            attn_t = [[None] * NCOL for _ in range(NL)]
            with ExitStack() as p2:
                kpool = p2.enter_context(tc.tile_pool(name="kf", bufs=KL))
                vpool = p2.enter_context(tc.tile_pool(name="vall", bufs=NTT))
                qpool = p2.enter_context(tc.tile_pool(name="qf", bufs=3))
                epool = p2.enter_context(tc.tile_pool(name="ep", bufs=10))
                spool = p2.enter_context(tc.tile_pool(name="small2", bufs=4))
                psum2 = p2.enter_context(tc.tile_pool(name="psum2", bufs=2, space="PSUM"))

                kfs = []
                for kv in range(KL):
                    kf_t = kpool.tile([128, T], F32R, tag="kft")
                    nc.sync.dma_start(kf_t, kf_d[kv])
                    kfs.append(kf_t)
                vall = []
                for tt in range(NTT):
                    v_t = vpool.tile([128, KL * 128], F32R, tag="vt")
                    nc.sync.dma_start(v_t, v_d[tt])
                    vall.append(v_t)

                for n in range(NL):
                    kv = n // 4
                    qf_t = qpool.tile([128, T], F32R, tag="qft")
                    nc.sync.dma_start(qf_t, qf_d[n])
                    gate_t = qpool.tile([128, T], F16, tag="gat")
                    nc.sync.dma_start(gate_t, gate_d[n])
                    for j in range(NCOL):
                        ns = 2 * j + 2
                        ps_pv = psum2.tile([128, TQ], F32, tag="ppv", bufs=2)
                        ps_de = psum2.tile([1, TQ], F32, tag="pde", bufs=1)
                        qcol = qf_t[:, j * TQ:(j + 1) * TQ]
                        for si in range(ns):
                            ps_sc = psum2.tile([128, TQ], F32, tag="psc", bufs=4)
                            nc.tensor.matmul(ps_sc, kfs[kv][:, si * 128:(si + 1) * 128],
                                             qcol, start=True, stop=True)
                            e_t = epool.tile([128, TQ], F32R, tag="et")
                            if si >= ns - 2:
                                er = epool.tile([128, TQ], F32, tag="er")
                                nc.scalar.activation(er, ps_sc, AF.Exp)
                                mt = c_m0 if si == ns - 2 else c_m1
                                nc.vector.tensor_mul(e_t, er, mt)
                            else:
                                nc.scalar.activation(e_t, ps_sc, AF.Exp)
                            nc.tensor.matmul(ps_de, c_onec, e_t,
                                             start=(si == 0), stop=(si == ns - 1))
                            nc.tensor.matmul(ps_pv, vall[si][:, kv * 128:(kv + 1) * 128],
                                             e_t, start=(si == 0), stop=(si == ns - 1))
                        rr2 = spool.tile([1, TQ], F32R, tag="rr2")
                        with nc.allow_low_precision(reason="f32r output is f32-width"):
                            nc.vector.reciprocal(rr2, ps_de)
                        ps_rb = psum2.tile([128, TQ], F32, tag="prb", bufs=1)
                        nc.tensor.matmul(ps_rb, c_oner, rr2, start=True, stop=True)
                        tmp = spool.tile([128, TQ], F32, tag="tmp")
                        nc.vector.tensor_mul(tmp, ps_pv, gate_t[:, j * TQ:(j + 1) * TQ])
                        at = apool.tile([128, TQ], F32R, tag="attn")
                        nc.vector.tensor_mul(at, tmp, ps_rb)
                        attn_t[n][j] = at

            if _PHASES < 3:
                raise _PhaseStop()
            # ================= PHASE 3: output projection =================
            with ExitStack() as p3:
                wopool = p3.enter_context(tc.tile_pool(name="wo", bufs=3))
                opool = p3.enter_context(tc.tile_pool(name="osb", bufs=8))
                psum3 = p3.enter_context(tc.tile_pool(name="psum3", bufs=NTT, space="PSUM"))
                for d in range(ND):
                    wo_t = wopool.tile([128, NL, 512], F32R, tag="wot")
                    nc.sync.dma_start(wo_t, wop[d])
                    pso = []
                    for tq in range(NTT):
                        po = psum3.tile([128, 512], F32, tag="pso")
                        pso.append(po)
                    for h in range(NL):
                        for tq in range(NTT):
                            at = attn_t[h][tq // 2]
                            lhsT = at[:, (tq % 2) * 128:(tq % 2 + 1) * 128]
                            nc.tensor.matmul(pso[tq], lhsT, wo_t[:, h, :],
                                             start=(h == 0), stop=(h == NL - 1))
                    for tq in range(NTT):
                        osb = opool.tile([128, 512], F32, tag="osb")
                        nc.scalar.activation(osb, pso[tq], AF.Copy)
                        nc.sync.dma_start(out[tq * 128:(tq + 1) * 128,
                                              d * 512:(d + 1) * 512], osb)

      except _PhaseStop:
        pass
    nc.compile()
    _NC_CACHE["nc"] = nc
    return nc


def _rope_tables(pos, norm_w):
    """cos [128,T] and sin [80,T] tables with (1+w) folded in."""
    pos = pos.astype(np.float32)
    fraction = (2.0 * np.arange(RH, dtype=np.float32) / np.float32(ROT))
    timescale = np.power(np.float32(THETA), fraction).astype(np.float32)
    ang = (pos[None, :] / timescale[:, None]).astype(np.float32)  # [40, T]
    cosv = np.cos(ang).astype(np.float32)
    sinv = np.sin(ang).astype(np.float32)
    w1 = 1.0 + norm_w.astype(np.float32)  # [128]
    cos_t = np.ones((128, pos.shape[0]), np.float32)
    cos_t[0:RH] = cosv
    cos_t[RH:ROT] = cosv
    cos_t *= w1[:, None]
    sin_t = np.empty((ROT, pos.shape[0]), np.float32)
    sin_t[0:RH] = -sinv * w1[RH:ROT, None]   # partner is h+40
    sin_t[RH:ROT] = sinv * w1[0:RH, None]    # partner is h-40
    return cos_t, sin_t


def make_in_maps(x, positions, wq, wk, wv, wo, q_norm_w, k_norm_w):
    in_maps = []
    pswap = np.zeros((128, 128), np.float32)
    for m in range(RH):
        pswap[m + RH, m] = 1.0
    for m in range(RH, ROT):
        pswap[m - RH, m] = 1.0
    f_idx = np.arange(TQ)[None, :]
    p_idx = np.arange(128)[:, None]
    masks = np.stack([(p_idx <= f_idx).astype(np.float32),
                      (p_idx + 128 <= f_idx).astype(np.float32)])
    onec = np.ones((128, 1), np.float32)
    oner = np.ones((1, 128), np.float32)

    for c in range(N_CORES):
        b, half = c // 2, c % 2
        n0, k0 = half * NL, half * KL
        xtp = np.ascontiguousarray(x[b].T).reshape(NKT, 128, T)
        wqp = np.ascontiguousarray(
            wq[:, n0:n0 + NL, :].reshape(NKT, 128, NL, 2, 128).transpose(2, 3, 1, 0, 4))
        wkp = np.ascontiguousarray(
            wk[:, k0:k0 + KL, :].reshape(NKT, 128, KL, 128).transpose(2, 1, 0, 3))
        wvp = np.ascontiguousarray(wv[:, k0:k0 + KL, :].reshape(NKT, 128, KL * 128))
        wop = np.ascontiguousarray(
            wo[n0:n0 + NL].reshape(NL, 128, ND, 512).transpose(2, 1, 0, 3))
        cq, sq = _rope_tables(positions[b], q_norm_w)
        ck, sk = _rope_tables(positions[b], k_norm_w)
        in_maps.append({
            "xtp": xtp, "wqp": wqp, "wkp": wkp, "wvp": wvp, "wop": wop,
            "cosq": cq, "sinq": sq, "cosk": ck, "sink": sk,
            "pswap": pswap, "masks": masks, "onec": onec, "oner": oner,
            "epsq": np.full((1, 1), H * EPS, np.float32),
            "epsk": np.full((1, 1), EPS, np.float32),
        })
    return in_maps


def _wait_devices_healthy(attempts=8, sleep_s=15):
    """The axon-tunneled devices occasionally report NRT_EXEC_UNIT_UNRECOVERABLE
    transiently (e.g. after an aborted process); they recover on retry."""
    import jax, time
    for attempt in range(attempts):
        try:
            jax.block_until_ready(
                [jax.device_put(np.ones(4, np.float32), d) + 1
                 for d in jax.devices()[:N_CORES]])
            return
        except Exception:
            if attempt == attempts - 1:
                raise
            time.sleep(sleep_s)


def kernel(x, positions, wq, wk, wv, wo, q_norm_w, k_norm_w):
    import time
    x = np.asarray(x, np.float32)
    positions = np.asarray(positions)
    wq = np.asarray(wq, np.float32)
    wk = np.asarray(wk, np.float32)
    wv = np.asarray(wv, np.float32)
    wo = np.asarray(wo, np.float32)
    q_norm_w = np.asarray(q_norm_w, np.float32)
    k_norm_w = np.asarray(k_norm_w, np.float32)

    nc = build_nc()
    in_maps = make_in_maps(x, positions, wq, wk, wv, wo, q_norm_w, k_norm_w)
    _wait_devices_healthy()
    res = None
    for attempt in range(3):
        try:
            res = run_bass_kernel_spmd(nc, in_maps, core_ids=list(range(N_CORES)))
            break
        except Exception:
            if attempt == 2:
                raise
            time.sleep(20)
            _wait_devices_healthy()
    out = np.empty((B, T, D), np.float32)
    for b in range(B):
        out[b] = res.results[2 * b]["out"] + res.results[2 * b + 1]["out"]
    return out

